# revision 6
# baseline (speedup 1.0000x reference)
"""Trainium2 Bass kernel for the Context-ComplEx scoring module.

Distribution over 8 NeuronCores:
  - attention / message-passing is data-parallel over the batch
    (128 rows per core, gathers from the full entity table),
  - the big [B, n_ent] score matmul is tensor-parallel over entities
    (12500 entities per core) with an AllGather of the per-core query
    vectors in between,
  - small rank-sized parameters are replicated (pre-fused on host).

While the AllGather runs, each core computes the "diagonal" score block
(its own batch rows x its own entity shard) from its local query tile,
so the collective is hidden behind useful matmul + rhs streaming work.

Self-contained: shapes hardcoded, no sibling imports.
"""

import numpy as np
import ml_dtypes

import concourse.bass as bass
import concourse.mybir as mybir
import concourse.tile as tile
from concourse import bacc, bass_utils
from concourse.masks import make_identity

N_ENT, N_REL, R, B, M, NC = 100000, 1000, 256, 1024, 50, 8
ES = N_ENT // NC          # 12500 entities per core
BS = B // NC              # 128 batch rows per core
EW = 500                  # entity tile width for the score matmul
NJ = ES // EW             # 25 entity tiles
F32 = mybir.dt.float32
BF16 = mybir.dt.bfloat16
I32 = mybir.dt.int32
AX = mybir.AxisListType
ALU = mybir.AluOpType
ACTF = mybir.ActivationFunctionType

TRACE = False  # set True (with the ntff hook installed) to profile
_cache = {}


def _build(n_ent=N_ENT, n_rel=N_REL, es=ES, ew=EW, stage=3):
    nj = es // ew
    assert nj * ew == es

    nc = bacc.Bacc("TRN2", target_bir_lowering=False, debug=False, num_devices=NC)

    x_d = nc.dram_tensor("x", [BS, 3], I32, kind="ExternalInput").ap()
    nb_d = nc.dram_tensor("nb", [BS, M], I32, kind="ExternalInput").ap()
    E_d = nc.dram_tensor("E", [n_ent, 2 * R], F32, kind="ExternalInput").ap()
    Erel_d = nc.dram_tensor("Erel", [n_rel, 2 * R], F32, kind="ExternalInput").ap()
    Ect_d = nc.dram_tensor("Ect", [2 * R, es], BF16, kind="ExternalInput").ap()
    Wre_d = nc.dram_tensor("Wre", [4 * R, R], F32, kind="ExternalInput").ap()
    Wim_d = nc.dram_tensor("Wim", [4 * R, R], F32, kind="ExternalInput").ap()
    W2re_d = nc.dram_tensor("W2re", [2 * R, R], F32, kind="ExternalInput").ap()
    W2im_d = nc.dram_tensor("W2im", [2 * R, R], F32, kind="ExternalInput").ap()
    Ucat_d = nc.dram_tensor("Ucat", [128, 3 * R], F32, kind="ExternalInput").ap()
    Bcat_d = nc.dram_tensor("Bcat", [128, 4 * R], F32, kind="ExternalInput").ap()
    Bg_d = nc.dram_tensor("Bg", [128, 1], F32, kind="ExternalInput").ap()

    scores_d = nc.dram_tensor("scores", [B, es], F32, kind="ExternalOutput").ap()
    scloc_d = nc.dram_tensor("scloc", [BS, es], F32, kind="ExternalOutput").ap()
    nlhs_d = nc.dram_tensor("nlhs", [BS, R], F32, kind="ExternalOutput").ap()
    nrel_d = nc.dram_tensor("nrel", [BS, R], F32, kind="ExternalOutput").ap()
    nrhs_d = nc.dram_tensor("nrhs", [BS, R], F32, kind="ExternalOutput").ap()
    ngec_d = nc.dram_tensor("ngec", [BS, R], F32, kind="ExternalOutput").ap()

    with tile.TileContext(nc) as tc:
      with tc.tile_pool(name="dram", bufs=1, space="DRAM") as dram:
        qT_loc = dram.tile([4 * 128, BS], BF16, name="qT_loc")
        qT_all = dram.tile(
            [NC * 4 * 128, BS], BF16, addr_space="Shared", name="qT_all"
        )
        Ect_v = Ect_d.rearrange("(c p) e -> p c e", p=128)

        # phase-B pools opened first so their SBUF does not overlap phase
        # A's (rhs streaming / local matmuls proceed concurrently with AG)
        with (
            tc.tile_pool(name="pb", bufs=1) as pb,
            tc.tile_pool(name="rhsp", bufs=3) as rhsp,
            tc.tile_pool(name="evp", bufs=4) as evp,
            tc.tile_pool(name="psb", bufs=4, space="PSUM") as psb,
        ):

            def score_block(lhsT_of_kc, rhs_t, out_rows, j, tag):
                """One [BS x ew] score tile: 4 accumulating matmuls + evict."""
                ps = psb.tile([BS, ew], F32, name=f"ps_{tag}", tag="ps")
                for kc in range(4):
                    nc.tensor.matmul(
                        out=ps[:], lhsT=lhsT_of_kc(kc), rhs=rhs_t[:, kc, :],
                        start=(kc == 0), stop=(kc == 3),
                    )
                ev = evp.tile([BS, ew], F32, name=f"ev_{tag}", tag="ev")
                nc.vector.tensor_copy(out=ev[:], in_=ps[:])
                nc.sync.dma_start(
                    out=out_rows[:, j * ew:(j + 1) * ew], in_=ev[:]
                )

            # ------------- phase A: attention (this core's 128 rows) ---------
            with (
                tc.tile_pool(name="pa", bufs=1) as pa,
                tc.tile_pool(name="pap", bufs=2, space="PSUM") as pap,
                tc.tile_pool(name="trp", bufs=2, space="PSUM") as trp,
            ):
                ident = pa.tile([128, 128], F32, name="ident")
                make_identity(nc, ident[:])

                def transp(src_ap, dst_slice, label):
                    pt = trp.tile([128, 128], F32, name=f"pt_{label}", tag="pt")
                    nc.tensor.transpose(out=pt[:], in_=src_ap, identity=ident[:])
                    nc.vector.tensor_copy(out=dst_slice, in_=pt[:])

                x_sb = pa.tile([BS, 3], I32, name="x_sb")
                nc.sync.dma_start(out=x_sb[:], in_=x_d[:])
                nb_sb = pa.tile([BS, M], I32, name="nb_sb")
                nc.sync.dma_start(out=nb_sb[:], in_=nb_d[:])

                Wre_sb = pa.tile([128, 8, R], F32, name="Wre_sb")
                nc.sync.dma_start(
                    out=Wre_sb[:], in_=Wre_d.rearrange("(c p) r -> p c r", p=128)
                )
                Wim_sb = pa.tile([128, 8, R], F32, name="Wim_sb")
                nc.sync.dma_start(
                    out=Wim_sb[:], in_=Wim_d.rearrange("(c p) r -> p c r", p=128)
                )
                W2re_sb = pa.tile([128, 4, R], F32, name="W2re_sb")
                nc.sync.dma_start(
                    out=W2re_sb[:], in_=W2re_d.rearrange("(c p) r -> p c r", p=128)
                )
                W2im_sb = pa.tile([128, 4, R], F32, name="W2im_sb")
                nc.sync.dma_start(
                    out=W2im_sb[:], in_=W2im_d.rearrange("(c p) r -> p c r", p=128)
                )
                Ucat_sb = pa.tile([128, 3 * R], F32, name="Ucat_sb")
                nc.sync.dma_start(out=Ucat_sb[:], in_=Ucat_d[:])
                Bcat_sb = pa.tile([128, 4 * R], F32, name="Bcat_sb")
                nc.sync.dma_start(out=Bcat_sb[:], in_=Bcat_d[:])
                Bg_sb = pa.tile([128, 1], F32, name="Bg_sb")
                nc.sync.dma_start(out=Bg_sb[:], in_=Bg_d[:])

                # gathers of the triple embeddings
                lhs_sb = pa.tile([BS, 2 * R], F32, name="lhs_sb")
                nc.gpsimd.indirect_dma_start(
                    out=lhs_sb[:], out_offset=None, in_=E_d[:],
                    in_offset=bass.IndirectOffsetOnAxis(ap=x_sb[:, 0:1], axis=0),
                )
                rel_sb = pa.tile([BS, 2 * R], F32, name="rel_sb")
                nc.gpsimd.indirect_dma_start(
                    out=rel_sb[:], out_offset=None, in_=Erel_d[:],
                    in_offset=bass.IndirectOffsetOnAxis(ap=x_sb[:, 1:2], axis=0),
                )
                rhs_sb = pa.tile([BS, 2 * R], F32, name="rhs_sb")
                nc.gpsimd.indirect_dma_start(
                    out=rhs_sb[:], out_offset=None, in_=E_d[:],
                    in_offset=bass.IndirectOffsetOnAxis(ap=x_sb[:, 2:3], axis=0),
                )

                # neighbor gather (one tile per neighbor slot: precise deps)
                nbE = [
                    pa.tile([BS, 2 * R], F32, name=f"nbE_{m}") for m in range(M)
                ]
                for m in range(M):
                    nc.gpsimd.indirect_dma_start(
                        out=nbE[m][:], out_offset=None, in_=E_d[:],
                        in_offset=bass.IndirectOffsetOnAxis(
                            ap=nb_sb[:, m:m + 1], axis=0
                        ),
                    )

                def norm_out(src0, src1, out_d, label):
                    s0 = pa.tile([BS, R], F32, name=f"nsq0_{label}", tag="nsq0")
                    nc.scalar.activation(out=s0[:], in_=src0, func=ACTF.Square)
                    s1 = pa.tile([BS, R], F32, name=f"nsq1_{label}", tag="nsq1")
                    nc.scalar.activation(out=s1[:], in_=src1, func=ACTF.Square)
                    nc.vector.tensor_add(out=s0[:], in0=s0[:], in1=s1[:])
                    nc.scalar.activation(out=s1[:], in_=s0[:], func=ACTF.Sqrt)
                    nc.sync.dma_start(out=out_d[:], in_=s1[:])

                norm_out(lhs_sb[:, 0:R], lhs_sb[:, R:2 * R], nlhs_d, "lhs")
                norm_out(rel_sb[:, 0:R], rel_sb[:, R:2 * R], nrel_d, "rel")
                norm_out(rhs_sb[:, 0:R], rhs_sb[:, R:2 * R], nrhs_d, "rhs")

                # triple embedding: tcat = [l0, r0, l1, r1] transposed
                tcatT = pa.tile([128, 8, BS], F32, name="tcatT")
                srcs = [
                    lhs_sb[:, 0:128], lhs_sb[:, 128:256],
                    rel_sb[:, 0:128], rel_sb[:, 128:256],
                    lhs_sb[:, 256:384], lhs_sb[:, 384:512],
                    rel_sb[:, 256:384], rel_sb[:, 384:512],
                ]
                for c, src in enumerate(srcs):
                    transp(src, tcatT[:, c, :], f"t{c}")

                pwre = pap.tile([BS, R], F32, name="pwre", tag="pw")
                for c in range(8):
                    nc.tensor.matmul(
                        out=pwre[:], lhsT=tcatT[:, c, :], rhs=Wre_sb[:, c, :],
                        start=(c == 0), stop=(c == 7),
                    )
                pwim = pap.tile([BS, R], F32, name="pwim", tag="pw")
                for c in range(8):
                    nc.tensor.matmul(
                        out=pwim[:], lhsT=tcatT[:, c, :], rhs=Wim_sb[:, c, :],
                        start=(c == 0), stop=(c == 7),
                    )
                # wcat = [w0, -w1]; Bcat slot0 = bw0, slot1 = -bw1
                wcat = pa.tile([BS, 2 * R], F32, name="wcat")
                nc.vector.tensor_add(
                    out=wcat[:, 0:R], in0=pwre[:], in1=Bcat_sb[:, 0:R]
                )
                nc.vector.scalar_tensor_tensor(
                    out=wcat[:, R:2 * R], in0=pwim[:], scalar=-1.0,
                    in1=Bcat_sb[:, R:2 * R], op0=ALU.mult, op1=ALU.add,
                )

                # logits[b, m] = sum_k nbE[b, m, k] * wcat[b, k]
                logits = pa.tile([BS, M], F32, name="logits")
                scr = pa.tile([BS, 2 * R], F32, name="scr")
                for m in range(M):
                    nc.vector.scalar_tensor_tensor(
                        out=scr[:], in0=nbE[m][:], scalar=1.0,
                        in1=wcat[:], op0=ALU.mult, op1=ALU.mult,
                        accum_out=logits[:, m:m + 1],
                    )

                # softmax over m
                mxn = pa.tile([BS, 1], F32, name="mxn")
                nc.vector.reduce_max(
                    out=mxn[:], in_=logits[:], axis=AX.X, negate=True
                )
                alpha = pa.tile([BS, M], F32, name="alpha")
                ssum = pa.tile([BS, 1], F32, name="ssum")
                nc.scalar.activation(
                    out=alpha[:], in_=logits[:], func=ACTF.Exp,
                    bias=mxn[:, 0:1], accum_out=ssum[:],
                )
                rinv = pa.tile([BS, 1], F32, name="rinv")
                nc.vector.reciprocal(out=rinv[:], in_=ssum[:])
                nc.vector.tensor_scalar_mul(alpha[:], alpha[:], rinv[:, 0:1])

                # ec[b, k] = sum_m alpha[b, m] * nbE[b, m, k]  (ping-pong accum)
                ecA = pa.tile([BS, 2 * R], F32, name="ecA")
                ecB = pa.tile([BS, 2 * R], F32, name="ecB")
                nc.vector.tensor_scalar_mul(ecA[:], nbE[0][:], alpha[:, 0:1])
                cur, nxt = ecA, ecB
                for m in range(1, M):
                    nc.vector.scalar_tensor_tensor(
                        out=nxt[:], in0=nbE[m][:],
                        scalar=alpha[:, m:m + 1], in1=cur[:],
                        op0=ALU.mult, op1=ALU.add,
                    )
                    cur, nxt = nxt, cur
                ec = cur

                # ec0n/ec1n = ecat @ W2re/W2im + bias
                ecT = pa.tile([128, 4, BS], F32, name="ecT")
                for c in range(4):
                    transp(ec[:, 128 * c:128 * (c + 1)], ecT[:, c, :], f"e{c}")
                pe0 = pap.tile([BS, R], F32, name="pe0", tag="pw")
                for c in range(4):
                    nc.tensor.matmul(
                        out=pe0[:], lhsT=ecT[:, c, :], rhs=W2re_sb[:, c, :],
                        start=(c == 0), stop=(c == 3),
                    )
                ec0n = pa.tile([BS, R], F32, name="ec0n")
                nc.vector.tensor_add(
                    out=ec0n[:], in0=pe0[:], in1=Bcat_sb[:, 2 * R:3 * R]
                )
                pe1 = pap.tile([BS, R], F32, name="pe1", tag="pw")
                for c in range(4):
                    nc.tensor.matmul(
                        out=pe1[:], lhsT=ecT[:, c, :], rhs=W2im_sb[:, c, :],
                        start=(c == 0), stop=(c == 3),
                    )
                ec1n = pa.tile([BS, R], F32, name="ec1n")
                nc.vector.tensor_add(
                    out=ec1n[:], in0=pe1[:], in1=Bcat_sb[:, 3 * R:4 * R]
                )

                # gate
                s0 = pa.tile([BS, R], F32, name="s0")
                s1 = pa.tile([BS, R], F32, name="s1")
                At = pa.tile([BS, R], F32, name="At")
                B2 = pa.tile([BS, R], F32, name="B2")
                nc.vector.tensor_mul(s0[:], lhs_sb[:, 0:R], rel_sb[:, 0:R])
                nc.vector.tensor_mul(s1[:], lhs_sb[:, R:2 * R], rel_sb[:, R:2 * R])
                nc.vector.tensor_sub(At[:], s0[:], s1[:])
                nc.vector.tensor_mul(s0[:], lhs_sb[:, R:2 * R], rel_sb[:, 0:R])
                nc.vector.tensor_mul(s1[:], lhs_sb[:, 0:R], rel_sb[:, R:2 * R])
                nc.vector.tensor_add(B2[:], s0[:], s1[:])

                g0 = pa.tile([BS, 1], F32, name="g0")
                nc.vector.scalar_tensor_tensor(
                    out=scr[:, 0:R], in0=At[:], scalar=1.0,
                    in1=Ucat_sb[:, 0:R],
                    op0=ALU.mult, op1=ALU.mult, accum_out=g0[:],
                )
                g1 = pa.tile([BS, 1], F32, name="g1")
                nc.vector.scalar_tensor_tensor(
                    out=scr[:, 0:R], in0=B2[:], scalar=1.0,
                    in1=Ucat_sb[:, R:2 * R],
                    op0=ALU.mult, op1=ALU.mult, accum_out=g1[:],
                )
                g2 = pa.tile([BS, 1], F32, name="g2")
                nc.vector.scalar_tensor_tensor(
                    out=scr[:, 0:R], in0=ec0n[:], scalar=1.0,
                    in1=Ucat_sb[:, 2 * R:3 * R],
                    op0=ALU.mult, op1=ALU.mult, accum_out=g2[:],
                )
                nc.vector.tensor_add(out=g0[:], in0=g0[:], in1=g1[:])
                nc.vector.tensor_add(out=g0[:], in0=g0[:], in1=g2[:])
                g = pa.tile([BS, 1], F32, name="g")
                nc.scalar.activation(
                    out=g[:], in_=g0[:], func=ACTF.Sigmoid, bias=Bg_sb[:, 0:1]
                )
                omg = pa.tile([BS, 1], F32, name="omg")
                nc.vector.tensor_scalar(
                    out=omg[:], in0=g[:], scalar1=-1.0, scalar2=1.0,
                    op0=ALU.mult, op1=ALU.add,
                )
                gec0 = pa.tile([BS, R], F32, name="gec0")
                nc.vector.tensor_scalar(
                    out=gec0[:], in0=ec0n[:], scalar1=g[:, 0:1],
                    scalar2=omg[:, 0:1], op0=ALU.mult, op1=ALU.add,
                )
                gec1 = pa.tile([BS, R], F32, name="gec1")
                nc.vector.tensor_scalar_mul(gec1[:], ec1n[:], g[:, 0:1])

                norm_out(gec0[:], gec1[:], ngec_d, "gec")

                # q = [A*gec0 + B*gec1, B*gec0 - A*gec1]
                qcat = pa.tile([BS, 2 * R], F32, name="qcat")
                nc.vector.tensor_mul(s0[:], At[:], gec0[:])
                nc.vector.tensor_mul(s1[:], B2[:], gec1[:])
                nc.vector.tensor_add(qcat[:, 0:R], s0[:], s1[:])
                nc.vector.tensor_mul(s0[:], B2[:], gec0[:])
                nc.vector.tensor_mul(s1[:], At[:], gec1[:])
                nc.vector.tensor_sub(qcat[:, R:2 * R], s0[:], s1[:])

                # qT (bf16) -> DRAM for the AllGather
                qT_bf = pa.tile([128, 4, BS], BF16, name="qT_bf")
                for c in range(4):
                    transp(qcat[:, 128 * c:128 * (c + 1)], qT_bf[:, c, :], f"q{c}")
                nc.sync.dma_start(
                    out=qT_loc.rearrange("(c p) b -> p c b", p=128), in_=qT_bf[:]
                )

                # sweep 1: local diagonal block (runs while the AG is in
                # flight; redundant with sweep 2 but fills the bubble)
                if stage >= 3:
                    for j in range(nj):
                        rl = rhsp.tile(
                            [128, 4, ew], BF16, name=f"rl_{j}", tag="rhs"
                        )
                        nc.sync.dma_start(
                            out=rl[:], in_=Ect_v[:, :, j * ew:(j + 1) * ew]
                        )
                        score_block(
                            lambda kc: qT_bf[:, kc, :],
                            rl, scloc_d, j, f"l{j}",
                        )

            if stage >= 2:
                nc.gpsimd.collective_compute(
                    "AllGather",
                    ALU.bypass,
                    replica_groups=[list(range(NC))],
                    ins=[qT_loc.opt()],
                    outs=[qT_all.opt()],
                )

            # ------------- phase B sweep 2: all batch chunks -----------------
            if stage >= 3:
                qa = pb.tile([128, NC * 4, BS], BF16, name="qa")
                nc.sync.dma_start(
                    out=qa[:], in_=qT_all.rearrange("(g p) b -> p g b", p=128)
                )
                for j in range(nj):
                    rhs_t = rhsp.tile(
                        [128, 4, ew], BF16, name=f"rhs_{j}", tag="rhs"
                    )
                    nc.sync.dma_start(
                        out=rhs_t[:], in_=Ect_v[:, :, j * ew:(j + 1) * ew]
                    )
                    for bc in range(NC):
                        score_block(
                            lambda kc, bc=bc: qa[:, bc * 4 + kc, :],
                            rhs_t,
                            scores_d[bc * BS:(bc + 1) * BS, :],
                            j, f"{j}_{bc}",
                        )

    nc.compile()
    return nc


def _prep(inputs):
    bf16 = ml_dtypes.bfloat16
    f32 = np.float32

    def arr(name, dtype=f32):
        return np.ascontiguousarray(np.asarray(inputs[name]), dtype=dtype)

    x = arr("x", np.int32)
    nb = arr("nb_idx", np.int32)
    E = arr("E_ent")
    Erel = arr("E_rel")
    W0, W1 = arr("W0"), arr("W1")
    bw0, bw1 = arr("bw0"), arr("bw1")
    W20, W21 = arr("W20"), arr("W21")
    bw20, bw21 = arr("bw20"), arr("bw21")
    Uo0, Uo1, Wo0, b_g = arr("Uo0"), arr("Uo1"), arr("Wo0"), arr("b_g")

    Wre = np.concatenate([W0, -W1], axis=0)
    Wim = np.concatenate([W1, W0], axis=0)
    W2re = np.concatenate([W20, -W21], axis=0)
    W2im = np.concatenate([W21, W20], axis=0)
    Ucat = np.concatenate(
        [
            np.tile(Uo0.reshape(1, R), (128, 1)),
            np.tile(-Uo1.reshape(1, R), (128, 1)),
            np.tile(Wo0.reshape(1, R), (128, 1)),
        ],
        axis=1,
    ).astype(f32)
    Bcat = np.concatenate(
        [
            np.tile(bw0.reshape(1, R), (128, 1)),
            np.tile(-bw1.reshape(1, R), (128, 1)),
            np.tile(bw20.reshape(1, R), (128, 1)),
            np.tile(bw21.reshape(1, R), (128, 1)),
        ],
        axis=1,
    ).astype(f32)
    Bg = np.full((128, 1), float(b_g.reshape(-1)[0]), f32)

    Eb = E.astype(bf16)
    in_maps = []
    for c in range(NC):
        Ect = np.ascontiguousarray(Eb[c * ES:(c + 1) * ES, :].T)
        in_maps.append(
            {
                "x": np.ascontiguousarray(x[c * BS:(c + 1) * BS]),
                "nb": np.ascontiguousarray(nb[c * BS:(c + 1) * BS]),
                "E": E,
                "Erel": Erel,
                "Ect": Ect,
                "Wre": Wre,
                "Wim": Wim,
                "W2re": W2re,
                "W2im": W2im,
                "Ucat": Ucat,
                "Bcat": Bcat,
                "Bg": Bg,
            }
        )
    return in_maps


def kernel(**inputs):
    if "nc" not in _cache:
        _cache["nc"] = _build()
    nc = _cache["nc"]
    in_maps = _prep(inputs)
    res = bass_utils.run_bass_kernel_spmd(
        nc, in_maps, list(range(NC)), trace=TRACE
    )
    _cache["last_result"] = res
    out = res.results
    scores = np.concatenate([out[c]["scores"] for c in range(NC)], axis=1)
    n_lhs = np.concatenate([out[c]["nlhs"] for c in range(NC)], axis=0)
    n_rel = np.concatenate([out[c]["nrel"] for c in range(NC)], axis=0)
    n_rhs = np.concatenate([out[c]["nrhs"] for c in range(NC)], axis=0)
    n_gec = np.concatenate([out[c]["ngec"] for c in range(NC)], axis=0)
    return scores, n_lhs, n_rel, n_rhs, n_gec
